# revision 4
# baseline (speedup 1.0000x reference)
"""Trainium2 Bass kernel for BaseGenerator: mapped = mapping @ base_flat.

Strategy (8-core SPMD, pure data-parallel over output pixels):
  - mapping [P1=16384, P0=16384] f32 is row-sharded: core c owns output rows
    [c*2048, (c+1)*2048).  Host pre-transposes each shard to mt_c [P0, 2048]
    (K-major, cast to fp8 e4m3) so the contraction axis lands on SBUF
    partitions and the device streams the shard as a few large contiguous
    DMAs plus a fine-grained tail (so the final matmuls chase the stream).
  - base_flat [P0, 3] is replicated, rearranged host-side to [128, 128, 16]
    (3 fp8 values + 13 pad per K-chunk).
  - Device modes (COMPUTE_MODE):
      "ct": column-tiled fp8 -- per K-chunk, 4 concurrent matmuls on
            disjoint 32-column strips of the PE array (tile_position=(0,32g)),
            one per 512-pixel output block, each accumulating into its own
            partition slice [32g:32g+3] of one PSUM bank.  ~4x PE ingest
            vs a single stream, so the PE never gates the DMA stream even
            when the HAM clock-gate has it cold.
      "dr": fp8 DoubleRow -- each matmul contracts a pair of K-chunks
            ([128,2,512] moving AP) at 2 MACs/cell/cycle.  ~2x.
      "plain": one matmul per K-chunk (use with float16 for A/B tests).
  - Epilogue copies the four [3,512] PSUM slices -> SBUF (vector engine for
    the cross-quadrant moves, scalar for the aligned one) -> one DMA to
    DRAM out [3, 2048] f32; host concatenates -> [16384, 3] -> [128,128,3].

The kernel is DMA-bound: 32 MiB/core (fp8) streams at the per-core HBM
limit (~360-450 GB/s); fp8 e4m3 quantization of both operands lands at
absmax rel err ~1.5e-3 vs the f32 reference (tolerance 2e-2).
"""

import sys

import numpy as np

try:
    import concourse.bacc as bacc
except ImportError:  # fresh env without PYTHONPATH: fall back to repo paths
    for _p in ("/opt/trn_rl_repo", "/opt/pypackages",
               "/root/.axon_site/_ro/trn_rl_repo",
               "/root/.axon_site/_ro/pypackages"):
        if _p not in sys.path:
            sys.path.append(_p)
    import concourse.bacc as bacc
import concourse.bass as bass
import concourse.mybir as mybir
import concourse.tile as tile
from concourse.bass_utils import run_bass_kernel_spmd

H0 = W0 = 128
H1 = W1 = 128
P0 = H0 * W0          # 16384 contraction length
P1 = H1 * W1          # 16384 output pixels
N_CORES = 8
N_PER_CORE = P1 // N_CORES   # 2048 output pixels per core
KC = 128              # K-chunk size (SBUF partitions)
N_KCHUNKS = P0 // KC  # 128
NB = 512              # matmul moving free dim (one PSUM bank of f32)
N_BANKS = N_PER_CORE // NB   # 4
BPAD = 16             # bytes per K-chunk of base weights (3 used + pad)

COMPUTE_DTYPE = "float8e4"
COMPUTE_MODE = "ct"          # "ct" | "dr" | "plain"
CHUNKS_PER_DMA = 16          # K-chunks fetched per dma_start (even!)
DMA_BUFS = 4                 # in-flight DMA tiles
TAIL_CHUNKS = 2              # chunks per fine-grained tail piece (even!)

_PROGRAM_CACHE = {}


def _np_compute_dtype(name):
    import ml_dtypes
    if name == "float32":
        return np.float32
    if name == "float16":
        return np.float16
    if name == "float8e4":
        return ml_dtypes.float8_e4m3fn
    return ml_dtypes.bfloat16


def _build_program(dtype_name, mode):
    """Build + compile the SPMD Bass program (identical on all 8 cores)."""
    dt = getattr(mybir.dt, dtype_name)
    nc = bacc.Bacc(
        "TRN2", target_bir_lowering=False, debug=False, num_devices=N_CORES
    )
    qc = CHUNKS_PER_DMA
    n_dmas = N_KCHUNKS // qc
    mt = nc.dram_tensor("mt", [n_dmas * KC, qc, N_PER_CORE], dt,
                        kind="ExternalInput")
    bt = nc.dram_tensor("bt", [KC, N_KCHUNKS, BPAD], dt, kind="ExternalInput")
    out = nc.dram_tensor(
        "out", [3, N_PER_CORE], mybir.dt.float32, kind="ExternalOutput"
    )

    # mt[(i*KC)+p, a, n] holds mapping^T K-chunk (i*qc + a) so one DMA tile
    # is a contiguous qc*N_PER_CORE-byte read per partition.
    with tile.TileContext(nc) as tc:
        with (
            tc.tile_pool(name="bpool", bufs=1) as bpool,
            tc.tile_pool(name="mpool", bufs=DMA_BUFS) as mpool,
            tc.tile_pool(name="psum", bufs=1, space=bass.MemorySpace.PSUM) as pp,
            tc.tile_pool(name="opool", bufs=1) as opool,
        ):
            # First mt tile DMA is issued before bt so the big stream starts
            # as early as possible; bt (256 KB) lands well within tile 0.
            m_sb0 = mpool.tile([KC, qc, N_PER_CORE], dt, name="m_sb")
            nc.sync.dma_start(m_sb0[:], mt[0:KC])
            b_sb = bpool.tile([KC, N_KCHUNKS, BPAD], dt)
            nc.sync.dma_start(b_sb[:], bt[:])

            if mode == "ct":
                ps_all = pp.tile([128, NB], mybir.dt.float32, name="ps")
            else:
                ps = [
                    pp.tile([3, NB], mybir.dt.float32, name=f"ps{i}",
                            tag=f"ps{i}")
                    for i in range(N_BANKS)
                ]

            def chunk_mms(m_tile, k1, a):
                """Matmuls for K-chunk(s) starting at global chunk k1 =
                local chunk a of m_tile."""
                if mode == "ct":
                    lhsT = b_sb[:, k1:k1 + 1, 0:3]
                    for g in range(N_BANKS):
                        nc.tensor.matmul(
                            ps_all[32 * g:32 * g + 3, :],
                            lhsT,
                            m_tile[:, a:a + 1, g * NB:(g + 1) * NB],
                            start=(k1 == 0),
                            stop=(k1 == N_KCHUNKS - 1),
                            tile_position=(0, 32 * g),
                        )
                elif mode == "dr":
                    lhsT = b_sb[:, k1:k1 + 2, 0:3]
                    for nb in range(N_BANKS):
                        nc.tensor.matmul(
                            ps[nb][:, :],
                            lhsT,
                            m_tile[:, a:a + 2, nb * NB:(nb + 1) * NB],
                            start=(k1 == 0),
                            stop=(k1 == N_KCHUNKS - 2),
                            perf_mode=mybir.MatmulPerfMode.DoubleRow,
                        )
                else:
                    lhsT = b_sb[:, k1:k1 + 1, 0:3]
                    for nb in range(N_BANKS):
                        nc.tensor.matmul(
                            ps[nb][:, :],
                            lhsT,
                            m_tile[:, a:a + 1, nb * NB:(nb + 1) * NB],
                            start=(k1 == 0),
                            stop=(k1 == N_KCHUNKS - 1),
                        )

            kstep = 2 if mode == "dr" else 1

            # Main stream: big DMA tiles for all but the last qc chunks.
            for i in range(n_dmas - 1):
                m_sb = m_sb0 if i == 0 else mpool.tile(
                    [KC, qc, N_PER_CORE], dt, name="m_sb"
                )
                if i > 0:
                    nc.sync.dma_start(m_sb[:], mt[i * KC:(i + 1) * KC])
                for a in range(0, qc, kstep):
                    chunk_mms(m_sb, i * qc + a, a)

            # Tail: last qc chunks arrive in small pieces so the PE's final
            # matmuls start as soon as each piece lands instead of waiting
            # for a whole qc-chunk DMA.
            tq = TAIL_CHUNKS
            for j in range(qc // tq):
                k_base = (n_dmas - 1) * qc + j * tq
                m_tl = mpool.tile(
                    [KC, tq, N_PER_CORE], dt, name="m_tl", tag="m_tl",
                    bufs=qc // tq,
                )
                nc.sync.dma_start(
                    m_tl[:],
                    mt[(n_dmas - 1) * KC:n_dmas * KC, j * tq:(j + 1) * tq],
                )
                for a in range(0, tq, kstep):
                    chunk_mms(m_tl, k_base + a, a)

            # Epilogue: PSUM -> SBUF, then one DMA.  In ct mode group g's
            # result lives at PSUM partitions [32g:32g+3]; the vector engine
            # handles the quadrant-aligned cross-partition moves (g>0).
            o_sb = opool.tile([3, N_PER_CORE], mybir.dt.float32)
            for nb in range(N_BANKS):
                dst = o_sb[:, nb * NB:(nb + 1) * NB]
                if mode == "ct":
                    src = ps_all[32 * nb:32 * nb + 3, :]
                    if nb == 0:
                        nc.scalar.copy(dst, src)
                    else:
                        nc.vector.tensor_copy(dst, src)
                else:
                    src = ps[nb][:, :]
                    if nb % 2 == 0:
                        nc.vector.tensor_copy(dst, src)
                    else:
                        nc.scalar.copy(dst, src)
            nc.sync.dma_start(out[:], o_sb[:])

    nc.compile()
    return nc


def _get_program(dtype_name, mode=None):
    mode = mode or COMPUTE_MODE
    key = (dtype_name, mode)
    if key not in _PROGRAM_CACHE:
        _PROGRAM_CACHE[key] = _build_program(dtype_name, mode)
    return _PROGRAM_CACHE[key]


def _prepare_inputs(mapping, base_image, dtype_name):
    np_dt = _np_compute_dtype(dtype_name)
    # base [128,128,3] -> base_flat [P0, 3] -> bt [128 part, 128 kchunk, 16]
    # bt[p, k1, c] = base_flat[k1*128 + p, c] for c < 3, 0-padded to 16.
    base_flat = np.asarray(base_image, dtype=np.float32).reshape(P0, 3)
    bt = np.zeros((KC, N_KCHUNKS, BPAD), dtype=np_dt)
    bt[:, :, 0:3] = base_flat.reshape(N_KCHUNKS, KC, 3).transpose(1, 0, 2)

    qc = CHUNKS_PER_DMA
    n_t = N_KCHUNKS // qc
    in_maps = []
    for c in range(N_CORES):
        shard = mapping[c * N_PER_CORE:(c + 1) * N_PER_CORE, :]  # [2048, P0] view
        mt_c = shard.T.astype(np_dt)  # [P0, 2048] K-major
        # tile-major: [tile i][partition p][chunk a][n] so each DMA tile is
        # one contiguous qc*2048 B read per partition.
        mt_c = np.ascontiguousarray(
            mt_c.reshape(n_t, qc, KC, N_PER_CORE).swapaxes(1, 2)
        ).reshape(n_t * KC, qc, N_PER_CORE)
        in_maps.append({"mt": mt_c, "bt": bt})
    return in_maps


def _run(mapping, base_image, dtype_name, trace=False, mode=None):
    nc = _get_program(dtype_name, mode)
    in_maps = _prepare_inputs(mapping, base_image, dtype_name)
    res = run_bass_kernel_spmd(nc, in_maps, list(range(N_CORES)), trace=trace)
    mapped_flat = np.concatenate(
        [res.results[c]["out"].T for c in range(N_CORES)], axis=0
    )  # [P1, 3] f32
    mapped_image = mapped_flat.reshape(H1, W1, 3)
    return mapped_image, res


def kernel(mapping, base_image):
    mapping = np.asarray(mapping, dtype=np.float32)
    base_image = np.asarray(base_image, dtype=np.float32)
    mapped_image, _ = _run(mapping, base_image, COMPUTE_DTYPE)
    return (base_image, mapped_image)


# revision 10
# speedup vs baseline: 1.1630x; 1.1630x over previous
"""Trainium2 Bass kernel for BaseGenerator: mapped = mapping @ base_flat.

Strategy (8-core SPMD, pure data-parallel over output pixels):
  - mapping [P1=16384, P0=16384] f32 is row-sharded: core c owns output rows
    [c*2048, (c+1)*2048).  Host pre-transposes each shard to mt_c [P0, 2048]
    (K-major, cast to fp8 e4m3) so the contraction axis lands on SBUF
    partitions and the device streams the shard as a few large contiguous
    DMAs plus a fine-grained tail (so the final matmuls chase the stream).
  - base_flat [P0, 3] is replicated, rearranged host-side to [128, 128, 16]
    (3 fp8 values + 13 pad per K-chunk).
  - Device modes (COMPUTE_MODE):
      "dr" (shipped): fp8 DoubleRow -- each matmul contracts a pair of
            K-chunks ([128,2,512] moving AP, [128,2,3] weight AP) at
            2 MACs/cell/cycle, 2x the PE ingest of a plain fp8 stream.
      "ct": column-tiled fp8 -- per K-chunk, 4 concurrent matmuls on
            disjoint 32-column strips of the PE array (tile_position=
            (0,32g)), one per 512-pixel output block.  4x PE ingest, but
            the ~1.2 TB/s SBUF read bursts slow the concurrent DMA writes
            (~12.2 us vs 9.9 us per 4 MiB tile measured), so "dr" is the
            better end-to-end config.
      "plain": one matmul per K-chunk (use with float16 for A/B tests).
  - Epilogue copies the four [3,512] PSUM banks -> SBUF on vector+scalar
    engines -> one DMA to DRAM out [3, 2048] f32; host concatenates the
    per-core outputs -> [16384, 3] -> [128, 128, 3].

The kernel is DMA-bound: 32 MiB/core (fp8) streams at the per-core HBM
limit (~430-455 GB/s in good phases, ~360 in bad; exec time is bimodal
~100/~116 us run-to-run).  ~13 us is fixed NEFF entry/exit tax (engine
check-in barrier + full semaphore-file zeroing) emitted by the compiler
wrapper.  fp8 e4m3 quantization of both operands lands at absmax rel err
~1.4e-3 vs the f32 reference (tolerance 2e-2).  Best measured: 100.4 us
(vs 180.7 us fp16 baseline).
"""

import sys

import numpy as np

try:
    import concourse.bacc as bacc
except ImportError:  # fresh env without PYTHONPATH: fall back to repo paths
    for _p in ("/opt/trn_rl_repo", "/opt/pypackages",
               "/root/.axon_site/_ro/trn_rl_repo",
               "/root/.axon_site/_ro/pypackages"):
        if _p not in sys.path:
            sys.path.append(_p)
    import concourse.bacc as bacc
import concourse.bass as bass
import concourse.mybir as mybir
import concourse.tile as tile
from concourse.bass_utils import run_bass_kernel_spmd

H0 = W0 = 128
H1 = W1 = 128
P0 = H0 * W0          # 16384 contraction length
P1 = H1 * W1          # 16384 output pixels
N_CORES = 8
N_PER_CORE = P1 // N_CORES   # 2048 output pixels per core
KC = 128              # K-chunk size (SBUF partitions)
N_KCHUNKS = P0 // KC  # 128
NB = 512              # matmul moving free dim (one PSUM bank of f32)
N_BANKS = N_PER_CORE // NB   # 4
BPAD = 16             # bytes per K-chunk of base weights (3 used + pad)

COMPUTE_DTYPE = "float8e4"
COMPUTE_MODE = "dr"          # "dr" | "ct" | "plain"
CHUNKS_PER_DMA = 8           # K-chunks fetched per dma_start (even!)
DMA_BUFS = 8                 # in-flight DMA tiles
TAIL_PLAN = (4, 2, 2)        # chunk counts of the fine-grained tail pieces
                             # (must sum to <= CHUNKS_PER_DMA; each even)
SCALAR_COPIES = 1            # how many epilogue PSUM copies go to scalar

_PROGRAM_CACHE = {}


def _np_compute_dtype(name):
    import ml_dtypes
    if name == "float32":
        return np.float32
    if name == "float16":
        return np.float16
    if name == "float8e4":
        return ml_dtypes.float8_e4m3fn
    return ml_dtypes.bfloat16


def _build_program(dtype_name, mode):
    """Build + compile the SPMD Bass program (identical on all 8 cores)."""
    dt = getattr(mybir.dt, dtype_name)
    nc = bacc.Bacc(
        "TRN2", target_bir_lowering=False, debug=False, num_devices=N_CORES
    )
    qc = CHUNKS_PER_DMA
    n_dmas = N_KCHUNKS // qc
    mt = nc.dram_tensor("mt", [n_dmas * KC, qc, N_PER_CORE], dt,
                        kind="ExternalInput")
    bt = nc.dram_tensor("bt", [KC, N_KCHUNKS, BPAD], dt, kind="ExternalInput")
    out = nc.dram_tensor(
        "out", [3, N_PER_CORE], mybir.dt.float32, kind="ExternalOutput"
    )

    # mt[(i*KC)+p, a, n] holds mapping^T K-chunk (i*qc + a) so one DMA tile
    # is a contiguous qc*N_PER_CORE-byte read per partition.
    with tile.TileContext(nc) as tc:
        with (
            tc.tile_pool(name="bpool", bufs=1) as bpool,
            tc.tile_pool(name="mpool", bufs=DMA_BUFS) as mpool,
            tc.tile_pool(name="psum", bufs=1, space=bass.MemorySpace.PSUM) as pp,
            tc.tile_pool(name="opool", bufs=1) as opool,
        ):
            # First mt tile DMA is issued before bt so the big stream starts
            # as early as possible; bt (256 KB) lands well within tile 0.
            m_sb0 = mpool.tile([KC, qc, N_PER_CORE], dt, name="m_sb")
            nc.sync.dma_start(m_sb0[:], mt[0:KC])
            b_sb = bpool.tile([KC, N_KCHUNKS, BPAD], dt)
            nc.sync.dma_start(b_sb[:], bt[:])

            if mode == "ct":
                ps_all = pp.tile([128, NB], mybir.dt.float32, name="ps")
            else:
                ps = [
                    pp.tile([3, NB], mybir.dt.float32, name=f"ps{i}",
                            tag=f"ps{i}")
                    for i in range(N_BANKS)
                ]

            def chunk_mms(m_tile, k1, a):
                """Matmuls for K-chunk(s) starting at global chunk k1 =
                local chunk a of m_tile."""
                if mode == "ct":
                    lhsT = b_sb[:, k1:k1 + 1, 0:3]
                    for g in range(N_BANKS):
                        nc.tensor.matmul(
                            ps_all[32 * g:32 * g + 3, :],
                            lhsT,
                            m_tile[:, a:a + 1, g * NB:(g + 1) * NB],
                            start=(k1 == 0),
                            stop=(k1 == N_KCHUNKS - 1),
                            tile_position=(0, 32 * g),
                        )
                elif mode == "dr":
                    lhsT = b_sb[:, k1:k1 + 2, 0:3]
                    for nb in range(N_BANKS):
                        nc.tensor.matmul(
                            ps[nb][:, :],
                            lhsT,
                            m_tile[:, a:a + 2, nb * NB:(nb + 1) * NB],
                            start=(k1 == 0),
                            stop=(k1 == N_KCHUNKS - 2),
                            perf_mode=mybir.MatmulPerfMode.DoubleRow,
                        )
                else:
                    lhsT = b_sb[:, k1:k1 + 1, 0:3]
                    for nb in range(N_BANKS):
                        nc.tensor.matmul(
                            ps[nb][:, :],
                            lhsT,
                            m_tile[:, a:a + 1, nb * NB:(nb + 1) * NB],
                            start=(k1 == 0),
                            stop=(k1 == N_KCHUNKS - 1),
                        )

            kstep = 2 if mode == "dr" else 1

            # Main stream: big DMA tiles for all but the last qc chunks.
            for i in range(n_dmas - 1):
                m_sb = m_sb0 if i == 0 else mpool.tile(
                    [KC, qc, N_PER_CORE], dt, name="m_sb"
                )
                if i > 0:
                    nc.sync.dma_start(m_sb[:], mt[i * KC:(i + 1) * KC])
                for a in range(0, qc, kstep):
                    chunk_mms(m_sb, i * qc + a, a)

            # Tail: last qc chunks arrive in progressively smaller pieces so
            # the PE's final matmuls chase the stream (the post-last-byte PE
            # work is only the final small piece) without paying small-
            # descriptor DMA rates for the whole tail.
            plan = list(TAIL_PLAN)
            head = qc - sum(plan)
            assert head >= 0 and all(t % kstep == 0 for t in plan)
            if head:
                plan = [head] + plan
            off = 0
            for j, tq in enumerate(plan):
                k_base = (n_dmas - 1) * qc + off
                m_tl = mpool.tile(
                    [KC, tq, N_PER_CORE], dt, name=f"m_tl{j}",
                    tag=f"m_tl{j}", bufs=1,
                )
                nc.sync.dma_start(
                    m_tl[:],
                    mt[(n_dmas - 1) * KC:n_dmas * KC, off:off + tq],
                )
                for a in range(0, tq, kstep):
                    chunk_mms(m_tl, k_base + a, a)
                off += tq

            # Epilogue: PSUM -> SBUF, then one DMA.  In ct mode group g's
            # result lives at PSUM partitions [32g:32g+3]; the vector engine
            # handles the quadrant-aligned cross-partition moves (g>0).
            o_sb = opool.tile([3, N_PER_CORE], mybir.dt.float32)
            for nb in range(N_BANKS):
                dst = o_sb[:, nb * NB:(nb + 1) * NB]
                if mode == "ct":
                    src = ps_all[32 * nb:32 * nb + 3, :]
                    if (nb % 2 == 0 and nb // 2 < SCALAR_COPIES):
                        nc.scalar.copy(dst, src)
                    else:
                        nc.vector.tensor_copy(dst, src)
                else:
                    src = ps[nb][:, :]
                    if nb % 2 == 0:
                        nc.vector.tensor_copy(dst, src)
                    else:
                        nc.scalar.copy(dst, src)
            nc.sync.dma_start(out[:], o_sb[:])

    nc.compile()
    return nc


def _get_program(dtype_name, mode=None):
    mode = mode or COMPUTE_MODE
    key = (dtype_name, mode)
    if key not in _PROGRAM_CACHE:
        _PROGRAM_CACHE[key] = _build_program(dtype_name, mode)
    return _PROGRAM_CACHE[key]


def _prepare_inputs(mapping, base_image, dtype_name):
    np_dt = _np_compute_dtype(dtype_name)
    # base [128,128,3] -> base_flat [P0, 3] -> bt [128 part, 128 kchunk, 16]
    # bt[p, k1, c] = base_flat[k1*128 + p, c] for c < 3, 0-padded to 16.
    base_flat = np.asarray(base_image, dtype=np.float32).reshape(P0, 3)
    bt = np.zeros((KC, N_KCHUNKS, BPAD), dtype=np_dt)
    bt[:, :, 0:3] = base_flat.reshape(N_KCHUNKS, KC, 3).transpose(1, 0, 2)

    qc = CHUNKS_PER_DMA
    n_t = N_KCHUNKS // qc
    in_maps = []
    for c in range(N_CORES):
        shard = mapping[c * N_PER_CORE:(c + 1) * N_PER_CORE, :]  # [2048, P0] view
        mt_c = shard.T.astype(np_dt)  # [P0, 2048] K-major
        # tile-major: [tile i][partition p][chunk a][n] so each DMA tile is
        # one contiguous qc*2048 B read per partition.
        mt_c = np.ascontiguousarray(
            mt_c.reshape(n_t, qc, KC, N_PER_CORE).swapaxes(1, 2)
        ).reshape(n_t * KC, qc, N_PER_CORE)
        in_maps.append({"mt": mt_c, "bt": bt})
    return in_maps


def _run(mapping, base_image, dtype_name, trace=False, mode=None):
    nc = _get_program(dtype_name, mode)
    in_maps = _prepare_inputs(mapping, base_image, dtype_name)
    res = run_bass_kernel_spmd(nc, in_maps, list(range(N_CORES)), trace=trace)
    mapped_flat = np.concatenate(
        [res.results[c]["out"].T for c in range(N_CORES)], axis=0
    )  # [P1, 3] f32
    mapped_image = mapped_flat.reshape(H1, W1, 3)
    return mapped_image, res


def kernel(mapping, base_image):
    mapping = np.asarray(mapping, dtype=np.float32)
    base_image = np.asarray(base_image, dtype=np.float32)
    mapped_image, _ = _run(mapping, base_image, COMPUTE_DTYPE)
    return (base_image, mapped_image)
